# revision 8
# baseline (speedup 1.0000x reference)
"""Trainium2 Bass kernel for nn_Attn_88691074662550.

Reference computation (jax):
    energy = enc @ W.T + b          # [S, H]
    scores = energy @ hidden        # [S]
    attn   = softmax(scores)        # [1, S]

Algebraic collapse used here:
    scores = enc @ (W.T @ hidden) + (b . hidden)
and softmax is shift-invariant, so the constant (b . hidden) drops out:
    attn = softmax(enc @ u),  u = W.T @ hidden

This is memory-bound: one streaming pass over the 256 MB encoder_outputs.
Sharding: encoder_outputs split along seq_len across 8 cores (32768 rows /
core); W and hidden replicated; softmax segmented per-partition + per-core,
with the 8 per-core (max, sum) pairs combined via a tiny AllGather on device.

Per-core layout: partition p owns rows [p*256, (p+1)*256) of the core's
shard, so every DMA moves contiguous per-partition byte runs.

Only standard BIR instructions are used (the walrus build in this
environment rejects bass_isa extended ISA ops like tensor_tensor_reduce /
partition_all_reduce with "ISA wrong length").
"""

import numpy as np

S = 262144
H = 256
NCORES = 8
SHARD = S // NCORES          # 32768 rows per core
P = 128                      # SBUF partitions
RPP = SHARD // P             # 256 rows per partition

_CACHE = {}


def _build(shard=SHARD, nchunk=16):
    """Build the Bass program (same program runs SPMD on all 8 cores)."""
    import concourse.bass as bass
    import concourse.tile as tile
    from concourse import mybir

    rpp = shard // P              # rows per partition
    nrc = rpp // nchunk           # rows per partition per chunk
    assert rpp % nchunk == 0
    f32 = mybir.dt.float32
    Alu = mybir.AluOpType
    Act = mybir.ActivationFunctionType
    Axis = mybir.AxisListType

    nc = bass.Bass(num_devices=NCORES)

    enc = nc.declare_dram_parameter("enc", [shard, H], f32, isOutput=False)
    w = nc.declare_dram_parameter("w", [H, H], f32, isOutput=False)
    hid = nc.declare_dram_parameter("hid", [1, H], f32, isOutput=False)
    attn = nc.declare_dram_parameter("attn", [1, shard], f32, isOutput=True)

    u_dram = nc.dram_tensor("u_dram", [1, H], f32)
    ms_dram = nc.dram_tensor("ms_dram", [1, 1], f32)

    def dram_bcast(ap, free_ap):
        """AP reading a DRAM region broadcast across all 128 partitions."""
        return bass.AP(tensor=ap.tensor, offset=ap.offset, ap=[[0, P]] + free_ap)

    with tile.TileContext(nc) as tc:
        with (
            tc.tile_pool(name="singles", bufs=1) as singles,
            tc.tile_pool(name="chunks", bufs=3) as chunks,
            tc.tile_pool(name="stats", bufs=1) as stats,
            tc.tile_pool(name="psum", bufs=1, space="PSUM") as psum,
            tc.tile_pool(name="dram", bufs=1, space="DRAM") as dram,
        ):
            # ---- u = W.T @ hidden, broadcast to all partitions ----
            # W rows k = kk*128 + p live at partition p, free slot kk.
            w_sb = singles.tile([P, 2, H], f32)
            nc.sync.dma_start(
                out=w_sb, in_=w[:].rearrange("(kk p) h -> p kk h", kk=2)
            )
            hid_sb = singles.tile([P, 2], f32)
            nc.gpsimd.dma_start(
                out=hid_sb, in_=hid[0, :].rearrange("(kk p) -> p kk", kk=2)
            )
            # Route both matmul inputs through DVE copies: the PE LoadWeights
            # slot only fits one sync wait, and the copies collapse the two
            # DMA-completion deps into a single DVE watermark.
            w_sb2 = singles.tile([P, 2, H], f32)
            nc.vector.tensor_copy(w_sb2, w_sb)
            hid_sb2 = singles.tile([P, 2], f32)
            nc.vector.tensor_copy(hid_sb2, hid_sb)
            psum_u = psum.tile([1, H], f32)
            for kk in range(2):
                nc.tensor.matmul(
                    out=psum_u,
                    lhsT=hid_sb2[:, kk : kk + 1],
                    rhs=w_sb2[:, kk, :],
                    start=(kk == 0),
                    stop=(kk == 1),
                )
            u_row = singles.tile([1, H], f32)
            nc.vector.tensor_copy(u_row, psum_u)
            nc.sync.dma_start(out=u_dram[:], in_=u_row)
            u_bc = singles.tile([P, H], f32)
            nc.gpsimd.dma_start(
                out=u_bc, in_=dram_bcast(u_dram[:], [[1, H]])
            )

            # ---- stream encoder shard; fused dot-product per row ----
            # scalar_tensor_tensor: out = (in0 bypass 0) * u ; accum = sum(out)
            scores = singles.tile([P, rpp], f32)
            dump = singles.tile([P, 1], f32)
            # Prime DVE's clock past the u_bc DMA so the first fused dot op
            # below needs only one sync wait (the chunk DMA) — the STT
            # instruction struct has a single wait slot in this codegen.
            nc.vector.tensor_copy(dump, u_bc[:, 0:1])
            enc_r = enc[:].rearrange("(p r) h -> p r h", p=P)
            for c in range(nchunk):
                xt = chunks.tile([P, nrc, H], f32, tag="xt")
                nc.sync.dma_start(out=xt, in_=enc_r[:, c * nrc : (c + 1) * nrc, :])
                for j in range(nrc):
                    col = c * nrc + j
                    nc.vector.scalar_tensor_tensor(
                        out=dump.broadcast_to((P, H)),
                        in0=xt[:, j, :],
                        scalar=0.0,
                        in1=u_bc,
                        op0=Alu.bypass,
                        op1=Alu.mult,
                        accum_out=scores[:, col : col + 1],
                    )

            # ---- segmented softmax: per-partition shift ----
            m_p = stats.tile([P, 1], f32)
            nc.vector.tensor_reduce(out=m_p, in_=scores, axis=Axis.X, op=Alu.max)
            neg_mp = stats.tile([P, 1], f32)
            nc.scalar.mul(out=neg_mp, in_=m_p, mul=-1.0)
            exp_sb = singles.tile([P, rpp], f32)
            s_p = stats.tile([P, 1], f32)
            nc.scalar.activation(
                out=exp_sb, in_=scores, func=Act.Exp, bias=neg_mp, scale=1.0,
                accum_out=s_p,
            )

            # ---- per-core (max, sum) via cross-partition reduces ----
            m1 = stats.tile([1, 1], f32)
            nc.gpsimd.tensor_reduce(out=m1, in_=m_p, axis=Axis.C, op=Alu.max)
            nc.sync.dma_start(out=ms_dram[:], in_=m1)
            m_bc = stats.tile([P, 1], f32)
            nc.gpsimd.dma_start(out=m_bc, in_=dram_bcast(ms_dram[:], [[1, 1]]))
            neg_mbc = stats.tile([P, 1], f32)
            nc.scalar.mul(out=neg_mbc, in_=m_bc, mul=-1.0)
            e_p = stats.tile([P, 1], f32)
            nc.scalar.activation(
                out=e_p, in_=m_p, func=Act.Exp, bias=neg_mbc, scale=1.0
            )
            w_p = stats.tile([P, 1], f32)
            nc.vector.tensor_mul(w_p, e_p, s_p)
            s1 = stats.tile([1, 1], f32)
            nc.gpsimd.tensor_reduce(out=s1, in_=w_p, axis=Axis.C, op=Alu.add)

            # ---- AllGather the 8 (max, sum) pairs ----
            pack = stats.tile([1, 2], f32)
            nc.vector.tensor_copy(pack[:, 0:1], m1)
            nc.vector.tensor_copy(pack[:, 1:2], s1)
            cc_in = dram.tile([1, 2], f32)
            cc_out = dram.tile([1, 2 * NCORES], f32)
            nc.gpsimd.dma_start(out=cc_in[:], in_=pack)
            nc.gpsimd.collective_compute(
                "AllGather",
                Alu.bypass,
                replica_groups=[list(range(NCORES))],
                ins=[cc_in[:]],
                outs=[cc_out[:]],
            )
            g = stats.tile([P, NCORES, 2], f32)
            nc.gpsimd.dma_start(
                out=g, in_=dram_bcast(cc_out[:], [[2, NCORES], [1, 2]])
            )

            # ---- global (max, sum); per-partition scale factor ----
            m_vec = g[:, :, 0]
            s_vec = g[:, :, 1]
            m_glob = stats.tile([P, 1], f32)
            nc.vector.tensor_reduce(out=m_glob, in_=m_vec, axis=Axis.X, op=Alu.max)
            neg_mg = stats.tile([P, 1], f32)
            nc.scalar.mul(out=neg_mg, in_=m_glob, mul=-1.0)
            t8 = stats.tile([P, NCORES], f32)
            nc.scalar.activation(
                out=t8, in_=m_vec, func=Act.Exp, bias=neg_mg, scale=1.0
            )
            z = stats.tile([P, 1], f32)
            dump8 = stats.tile([P, 1], f32)
            nc.vector.scalar_tensor_tensor(
                out=dump8.broadcast_to((P, NCORES)),
                in0=t8,
                scalar=0.0,
                in1=s_vec,
                op0=Alu.bypass,
                op1=Alu.mult,
                accum_out=z,
            )
            # alpha_p = exp(m_p - m_glob) / z
            e_a = stats.tile([P, 1], f32)
            nc.scalar.activation(
                out=e_a, in_=m_p, func=Act.Exp, bias=neg_mg, scale=1.0
            )
            rz = stats.tile([P, 1], f32)
            nc.vector.reciprocal(rz, z)
            alpha = stats.tile([P, 1], f32)
            nc.vector.tensor_mul(alpha, e_a, rz)

            # ---- final normalize and store ----
            final = singles.tile([P, rpp], f32)
            nc.vector.tensor_scalar_mul(final, exp_sb, alpha)
            nc.sync.dma_start(
                out=attn[0, :].rearrange("(p r) -> p r", p=P), in_=final
            )

    _split_excess_waits(nc, mybir)
    return nc


def _split_excess_waits(nc, mybir):
    """The walrus codegen here allows only one embedded sync wait on most
    instruction structs (STT, Matmult LW, ...). Spill extra waits into
    standalone EventSemaphore instructions placed just before, on the same
    engine — semantically identical, since all waits must pass before the
    instruction issues."""
    n = 0
    for fn in nc.m.functions:
        for blk in fn.blocks:
            out = []
            for inst in blk.instructions:
                si = inst.sync_info
                if (
                    si is not None
                    and si.on_wait
                    and len(si.on_wait) > 1
                    and inst.opcode not in ("EventSemaphore", "NoOp")
                ):
                    for w in si.on_wait[:-1]:
                        n += 1
                        ev = mybir.InstEventSemaphore(
                            name=f"EVSPILL-{n}", ins=[], outs=[]
                        )
                        ev.engine = inst.engine
                        ev.sync_info = mybir.SyncInfo(on_wait=[w], on_update=[])
                        out.append(ev)
                    si.on_wait = si.on_wait[-1:]
                out.append(inst)
            blk.instructions = out
    return nc


def _get_nc(shard=SHARD, nchunk=16):
    key = (shard, nchunk)
    if key not in _CACHE:
        _CACHE[key] = _build(shard, nchunk)
    return _CACHE[key]


def run(inputs, trace=False, shard=SHARD, nchunk=16):
    """Run on hardware. Returns (attn [1, S], BassKernelResults)."""
    from concourse.bass_utils import run_bass_kernel_spmd

    nc = _get_nc(shard, nchunk)
    enc_full = np.ascontiguousarray(inputs["encoder_outputs"], dtype=np.float32)
    w_full = np.ascontiguousarray(inputs["W"], dtype=np.float32)
    hid_full = np.ascontiguousarray(
        inputs["hidden"], dtype=np.float32
    ).reshape(1, H)
    n = enc_full.shape[0] // NCORES
    assert n == shard, f"expected shard {shard}, got {n}"
    in_maps = [
        {
            "enc": np.ascontiguousarray(enc_full[i * n : (i + 1) * n]),
            "w": w_full,
            "hid": hid_full,
        }
        for i in range(NCORES)
    ]
    res = run_bass_kernel_spmd(
        nc, in_maps, core_ids=list(range(NCORES)), trace=trace
    )
    out = np.concatenate([r["attn"] for r in res.results], axis=1)
    return out, res


def kernel(**inputs) -> np.ndarray:
    out, _ = run(inputs)
    return out
